# revision 32
# baseline (speedup 1.0000x reference)
"""GraphSAGE (2-layer, mean aggregation) on 8 Trainium2 NeuronCores.

Strategy:
  - Nodes are sharded contiguously across the 8 cores by destination row.
  - Aggregation (segment-mean over 800k edges) is done as: dma_gather of
    source-node features onto partitions (128 edges/chunk) and a
    TensorEngine matmul-accumulate into PSUM per 128-dst block, using a
    0/1 one-hot built ON-CHIP by the (otherwise idle) Vector engine via
    is_equal(iota, dst) — this removes ~60MB/core of one-hot HBM reads.
    The 1/deg mean scaling is applied once at PSUM drain time.
  - Hidden states are exchanged between layers with an AllGather
    collective (bf16, row-major) so layer-2 can gather any source row.
  - int16 gather indices can't address 50000 rows, so each block's edges
    are split into lo (src < 32768) and hi streams gathered from two
    slices of the feature table.
"""

import math
from contextlib import ExitStack

import numpy as np
import ml_dtypes

import concourse.bass as bass
import concourse.bacc as bacc
import concourse.mybir as mybir
import concourse.tile as tile
from concourse import bass_utils

P = 128
N_NODES = 50000
N_EDGES = 800000
D_IN = 128
D_HID = 128
D_OUT = 40
N_CORES = 8
LO_SPLIT = 32768          # int16 gather index limit boundary
GRP = 32                  # chunks per dma_gather call
GBUFS = 6                 # gather-tile double/triple buffering per stream
OBUFS = 4                 # on-chip one-hot tiles in flight
NQ = 4                    # swdge queues

BF16 = ml_dtypes.bfloat16


def _wrap_idxs(idx_flat):
    """dma_gather index layout: idx i lives at [i % 16, i // 16] of a
    16-partition tile, replicated to 128 partitions."""
    n = idx_flat.shape[0]
    assert n % 16 == 0
    w = idx_flat.reshape(n // 16, 16).T.astype(np.int16)  # [16, n/16]
    return np.tile(w, (8, 1))                             # [128, n/16]


def _balance_perm(src, dst, n_nodes, n_cores, lo_split):
    """Per-core row permutation that evens per-128-row-block lo/hi edge
    loads (reduces chunk padding). Returns perm: slot s holds node perm[s]."""
    rows_per = n_nodes // n_cores
    nblk = math.ceil(rows_per / P)
    lo_deg = np.bincount(dst[src < lo_split], minlength=n_nodes)
    hi_deg = np.bincount(dst[src >= lo_split], minlength=n_nodes)
    perm = np.empty(n_nodes, np.int64)
    for k in range(n_cores):
        r0 = k * rows_per
        rows = np.arange(r0, r0 + rows_per)
        ld, hd = lo_deg[rows].astype(np.float64), hi_deg[rows].astype(np.float64)
        order = np.argsort(-(ld + hd), kind="stable")
        cap = np.full(nblk, P, np.int64)
        cap[-1] = rows_per - (nblk - 1) * P
        lo_t = ld.sum() / rows_per * cap
        hi_t = hd.sum() / rows_per * cap
        lo_l = np.zeros(nblk)
        hi_l = np.zeros(nblk)
        cnt = np.zeros(nblk, np.int64)
        bins = [[] for _ in range(nblk)]
        for i in order:
            cost = np.maximum((lo_l + ld[i]) / lo_t, (hi_l + hd[i]) / hi_t)
            cost[cnt >= cap] = np.inf
            b = int(np.argmin(cost))
            bins[b].append(rows[i])
            lo_l[b] += ld[i]
            hi_l[b] += hd[i]
            cnt[b] += 1
        perm[r0 : r0 + rows_per] = np.concatenate(
            [np.asarray(b, np.int64) for b in bins]
        )
    return perm


def _gslot_idx(k, rows_per=N_NODES // N_CORES):
    """Global slot ids of core k's local rows, in local (block-major) order."""
    return np.arange(k * rows_per, (k + 1) * rows_per)


def preprocess(edge_index, n_nodes=N_NODES, n_cores=N_CORES, lo_split=LO_SPLIT):
    """Sort/partition edges; build per-core gather indices + per-chunk dst ids.

    Returns (meta, per_core) where per_core[k] holds the numpy arrays the
    device kernel consumes and meta holds the (uniform) structure sizes.
    The whole kernel works in permuted "slot" space (perm balances block
    loads); the host un-permutes the output rows at the end.
    """
    src0 = np.asarray(edge_index[0], dtype=np.int64)
    dst0 = np.asarray(edge_index[1], dtype=np.int64)
    perm = _balance_perm(src0, dst0, n_nodes, n_cores, lo_split)
    slot_of = np.empty(n_nodes, np.int64)
    slot_of[perm] = np.arange(n_nodes)
    src = slot_of[src0]
    dst = slot_of[dst0]
    counts = np.bincount(dst, minlength=n_nodes)
    inv_deg = (1.0 / np.maximum(counts, 1)).astype(np.float32)

    rows_per = n_nodes // n_cores
    nblk = math.ceil(rows_per / P)

    order = np.argsort(dst, kind="stable")
    s_s, d_s = src[order], dst[order]

    # boundaries of each (core, block) segment in the dst-sorted edge list
    blk_edges = {}
    n_lo_max, n_hi_max = 0, 0
    for k in range(n_cores):
        base = k * rows_per
        for b in range(nblk):
            r0 = base + b * P
            r1 = min(base + rows_per, r0 + P)
            e0 = np.searchsorted(d_s, r0, side="left")
            e1 = np.searchsorted(d_s, r1, side="left")
            s_seg, d_seg = s_s[e0:e1], d_s[e0:e1]
            lo_m = s_seg < lo_split
            blk_edges[(k, b)] = (s_seg, d_seg, lo_m, r0)
            n_lo_max = max(n_lo_max, int(lo_m.sum()))
            n_hi_max = max(n_hi_max, int((~lo_m).sum()))

    Llo = max(1, math.ceil(n_lo_max / P))
    Lhi = max(1, math.ceil(n_hi_max / P))
    C_lo, C_hi = nblk * Llo, nblk * Lhi

    per_core = []
    for k in range(n_cores):
        idx_lo = np.zeros((C_lo, P), np.int16)
        idx_hi = np.zeros((C_hi, P), np.int16)
        dst_lo = np.full((C_lo, P), -1.0, np.float32)
        dst_hi = np.full((C_hi, P), -1.0, np.float32)
        for b in range(nblk):
            s_seg, d_seg, lo_m, r0 = blk_edges[(k, b)]
            for (sel, idx_a, dst_a, L, off) in (
                (lo_m, idx_lo, dst_lo, Llo, 0),
                (~lo_m, idx_hi, dst_hi, Lhi, lo_split),
            ):
                ss = s_seg[sel] - off
                dd = d_seg[sel] - r0
                n = ss.shape[0]
                c0 = b * L
                fl_i = idx_a[c0 : c0 + L].reshape(-1)
                fl_d = dst_a[c0 : c0 + L].reshape(-1)
                fl_i[:n] = ss.astype(np.int16)
                fl_d[:n] = dd.astype(np.float32)

        per_core.append(
            dict(
                idx_lo=_wrap_idxs(idx_lo.reshape(-1)),
                idx_hi=_wrap_idxs(idx_hi.reshape(-1)),
                # [128 edge-slot partitions, C chunks]
                dstv_lo=np.ascontiguousarray(dst_lo.T).astype(BF16),
                dstv_hi=np.ascontiguousarray(dst_hi.T).astype(BF16),
                invdeg=np.tile(
                    inv_deg[_gslot_idx(k)][None, :], (P, 1)
                ).astype(BF16),
            )
        )

    meta = dict(
        n_nodes=n_nodes, n_cores=n_cores, rows_per=rows_per, nblk=nblk,
        Llo=Llo, Lhi=Lhi, C_lo=C_lo, C_hi=C_hi, lo_split=lo_split, perm=perm,
    )
    return meta, per_core


def build_graph(nc, m, d_in=D_IN, d_out=D_OUT):
    dt = mybir.dt
    alu = mybir.AluOpType
    act = mybir.ActivationFunctionType
    n_nodes, rows_per, nblk = m["n_nodes"], m["rows_per"], m["nblk"]
    Llo, Lhi, C_lo, C_hi = m["Llo"], m["Lhi"], m["C_lo"], m["C_hi"]
    lo_split = m["lo_split"]

    x_all = nc.dram_tensor("x_all", [n_nodes, d_in], dt.bfloat16, kind="ExternalInput")
    xT_d = nc.dram_tensor("xT", [P, rows_per], dt.bfloat16, kind="ExternalInput")
    idx_lo_d = nc.dram_tensor("idx_lo", [P, C_lo * 8], dt.int16, kind="ExternalInput")
    idx_hi_d = nc.dram_tensor("idx_hi", [P, C_hi * 8], dt.int16, kind="ExternalInput")
    dstv_lo_d = nc.dram_tensor("dstv_lo", [P, C_lo], dt.bfloat16, kind="ExternalInput")
    dstv_hi_d = nc.dram_tensor("dstv_hi", [P, C_hi], dt.bfloat16, kind="ExternalInput")
    invdeg_d = nc.dram_tensor("invdeg", [P, rows_per], dt.bfloat16, kind="ExternalInput")
    iota_d = nc.dram_tensor("iota", [P, P], dt.bfloat16, kind="ExternalInput")
    w1l_d = nc.dram_tensor("w1lT", [P, d_in], dt.bfloat16, kind="ExternalInput")
    w1r_d = nc.dram_tensor("w1rT", [P, d_in], dt.bfloat16, kind="ExternalInput")
    w2l_d = nc.dram_tensor("w2lT", [P, d_out], dt.bfloat16, kind="ExternalInput")
    w2r_d = nc.dram_tensor("w2rT", [P, d_out], dt.bfloat16, kind="ExternalInput")
    b1_d = nc.dram_tensor("b1r", [1, d_in], dt.bfloat16, kind="ExternalInput")
    b2_d = nc.dram_tensor("b2r", [1, d_out], dt.bfloat16, kind="ExternalInput")
    out_d = nc.dram_tensor("out", [rows_per, d_out], dt.float32, kind="ExternalOutput")

    with tile.TileContext(nc) as tc, ExitStack() as ctx:
        sb = ctx.enter_context(tc.tile_pool(name="sb", bufs=1))
        dram = ctx.enter_context(tc.tile_pool(name="dram", bufs=1, space="DRAM"))
        psum = ctx.enter_context(tc.tile_pool(name="psum", bufs=1, space="PSUM"))
        glo_p = ctx.enter_context(tc.tile_pool(name="glo", bufs=GBUFS))
        ghi_p = ctx.enter_context(tc.tile_pool(name="ghi", bufs=GBUFS))
        o_p = ctx.enter_context(tc.tile_pool(name="oh", bufs=OBUFS))
        st_p = ctx.enter_context(tc.tile_pool(name="st", bufs=2))

        def load(shape, dtype, src, name):
            t = sb.tile(shape, dtype, name=name)
            nc.sync.dma_start(t[:], src[:])
            return t

        xT_sb = load([P, rows_per], dt.bfloat16, xT_d.ap(), "xT_sb")
        idxlo_sb = load([P, C_lo * 8], dt.int16, idx_lo_d.ap(), "idxlo_sb")
        idxhi_sb = load([P, C_hi * 8], dt.int16, idx_hi_d.ap(), "idxhi_sb")
        dstlo_sb = load([P, C_lo], dt.bfloat16, dstv_lo_d.ap(), "dstlo_sb")
        dsthi_sb = load([P, C_hi], dt.bfloat16, dstv_hi_d.ap(), "dsthi_sb")
        invdeg_sb = load([P, rows_per], dt.bfloat16, invdeg_d.ap(), "invdeg_sb")
        iota_sb = load([P, P], dt.bfloat16, iota_d.ap(), "iota_sb")
        w1l_sb = load([P, d_in], dt.bfloat16, w1l_d.ap(), "w1l_sb")
        w1r_sb = load([P, d_in], dt.bfloat16, w1r_d.ap(), "w1r_sb")
        w2l_sb = load([P, d_out], dt.bfloat16, w2l_d.ap(), "w2l_sb")
        w2r_sb = load([P, d_out], dt.bfloat16, w2r_d.ap(), "w2r_sb")
        b1_sb = load([1, d_in], dt.bfloat16, b1_d.ap(), "b1_sb")
        b2_sb = load([1, d_out], dt.bfloat16, b2_d.ap(), "b2_sb")

        ones_sb = sb.tile([1, 512], dt.bfloat16, name="ones_sb")
        nc.vector.memset(ones_sb[:], 1.0)

        meanT = sb.tile([P, rows_per], dt.bfloat16, name="meanT")
        meanhT = sb.tile([P, rows_per], dt.bfloat16, name="meanhT")
        hT = sb.tile([P, rows_per], dt.bfloat16, name="hT")

        hsh = dram.tile([rows_per, d_in], dt.bfloat16, name="hsh")
        hfull = dram.tile([n_nodes, d_in], dt.bfloat16, name="hfull")

        qctr = [0]

        def aggregate(src_ap, outT):
            """outT[:, i] = (1/deg(i)) * sum_e src[srcnode(e), :] over edges
            into i. src rows gathered per edge; one-hot built on-chip."""
            streams = {
                "lo": dict(C=C_lo, idx=idxlo_sb, dstv=dstlo_sb,
                           ap=src_ap[0:lo_split, :], pool=glo_p, tag="glo"),
                "hi": dict(C=C_hi, idx=idxhi_sb, dstv=dsthi_sb,
                           ap=src_ap[lo_split:n_nodes, :], pool=ghi_p, tag="ghi"),
            }
            tiles = {}

            def ensure_group(stream, g):
                if (stream, g) in tiles:
                    return tiles[(stream, g)]
                s = streams[stream]
                c0, c1 = g * GRP, min(s["C"], (g + 1) * GRP)
                nch = c1 - c0
                n = nch * P
                t = s["pool"].tile([P, GRP, P], dt.bfloat16, tag=s["tag"],
                                   name=f"g_{s['tag']}")
                nc.gpsimd.dma_gather(
                    t[:, :nch, :], s["ap"],
                    s["idx"][:, c0 * 8 : c1 * 8],
                    n, n, d_in, elem_step=d_in, single_packet=False,
                    queue_num=qctr[0] % NQ,
                )
                qctr[0] += 1
                ot = o_p.tile([P, GRP, P], dt.bfloat16, tag="ohv", name="ohv")
                # build in quarters so early chunks' matmuls start sooner
                for h0 in range(0, nch, GRP // 4):
                    h1 = min(nch, h0 + GRP // 4)
                    nc.vector.tensor_tensor(
                        ot[:, h0:h1, :],
                        iota_sb[:, None, :].broadcast_to([P, h1 - h0, P]),
                        s["dstv"][:, c0 + h0 : c0 + h1, None].broadcast_to(
                            [P, h1 - h0, P]),
                        alu.is_equal,
                    )
                tiles[(stream, g)] = (t, ot)
                return tiles[(stream, g)]

            for b in range(nblk):
                bs = min(P, rows_per - b * P)
                ps = psum.tile([P, P], dt.float32, tag="agg", name="ps_agg",
                               bufs=4)
                ops = [("lo", c) for c in range(b * Llo, (b + 1) * Llo)]
                ops += [("hi", c) for c in range(b * Lhi, (b + 1) * Lhi)]
                for i, (stream, c) in enumerate(ops):
                    gt, ot = ensure_group(stream, c // GRP)
                    nc.tensor.matmul(
                        ps[:, :P], lhsT=gt[:, c % GRP, :], rhs=ot[:, c % GRP, :],
                        start=(i == 0), stop=(i == len(ops) - 1),
                    )
                nc.vector.tensor_tensor(
                    outT[:, b * P : b * P + bs], ps[:, :bs],
                    invdeg_sb[:, b * P : b * P + bs], alu.mult,
                )

        # ---- layer 1 ----
        aggregate(x_all.ap(), meanT)

        # row-major h (for the collective) first so the AllGather can start
        # while the hT panels below still run.
        for b in range(nblk):
            c0 = b * P
            bs = min(P, rows_per - c0)
            ps = psum.tile([P, 512], dt.float32, tag="ps", name="ps_r", bufs=4)
            nc.tensor.matmul(ps[:bs, :d_in], lhsT=meanT[:, c0 : c0 + bs], rhs=w1l_sb[:],
                             start=True, stop=False)
            nc.tensor.matmul(ps[:bs, :d_in], lhsT=xT_sb[:, c0 : c0 + bs], rhs=w1r_sb[:],
                             start=False, stop=False)
            nc.tensor.matmul(ps[:bs, :d_in], lhsT=ones_sb[:, :bs], rhs=b1_sb[:],
                             start=False, stop=True)
            hrow = st_p.tile([P, d_in], dt.bfloat16, tag="st", name="hrow")
            nc.scalar.activation(hrow[:bs, :], ps[:bs, :d_in], act.Relu)
            nc.sync.dma_start(hsh[c0 : c0 + bs, :], hrow[:bs, :])

        nc.gpsimd.collective_compute(
            "AllGather", alu.bypass,
            replica_groups=[list(range(m["n_cores"]))],
            ins=[hsh[:].opt()], outs=[hfull[:].opt()],
        )

        for c0 in range(0, rows_per, 512):
            w = min(512, rows_per - c0)
            ps = psum.tile([P, 512], dt.float32, tag="ps", name="ps_d", bufs=4)
            nc.tensor.matmul(ps[:, :w], lhsT=w1l_sb[:], rhs=meanT[:, c0 : c0 + w],
                             start=True, stop=False)
            nc.tensor.matmul(ps[:, :w], lhsT=w1r_sb[:], rhs=xT_sb[:, c0 : c0 + w],
                             start=False, stop=False)
            nc.tensor.matmul(ps[:, :w], lhsT=b1_sb[:], rhs=ones_sb[:, :w],
                             start=False, stop=True)
            nc.scalar.activation(hT[:, c0 : c0 + w], ps[:, :w], act.Relu)

        # ---- layer 2 ----
        aggregate(hfull, meanhT)

        for b in range(nblk):
            c0 = b * P
            bs = min(P, rows_per - c0)
            ps = psum.tile([P, 512], dt.float32, tag="ps", name="ps_o", bufs=4)
            nc.tensor.matmul(ps[:bs, :d_out], lhsT=meanhT[:, c0 : c0 + bs], rhs=w2l_sb[:],
                             start=True, stop=False)
            nc.tensor.matmul(ps[:bs, :d_out], lhsT=hT[:, c0 : c0 + bs], rhs=w2r_sb[:],
                             start=False, stop=False)
            nc.tensor.matmul(ps[:bs, :d_out], lhsT=ones_sb[:, :bs], rhs=b2_sb[:],
                             start=False, stop=True)
            ot = st_p.tile([P, d_out], dt.float32, tag="ot", name="ot")
            nc.vector.tensor_copy(ot[:bs, :], ps[:bs, :d_out])
            nc.sync.dma_start(out_d.ap()[c0 : c0 + bs, :], ot[:bs, :])

    return nc


def make_in_maps(inputs, meta, per_core):
    x = np.asarray(inputs["x"], np.float32)[meta["perm"]]
    n_cores, rows_per = meta["n_cores"], meta["rows_per"]
    x_bf = x.astype(BF16)
    w1l = np.asarray(inputs["W1l"], np.float32)
    w1r = np.asarray(inputs["W1r"], np.float32)
    w2l = np.asarray(inputs["W2l"], np.float32)
    w2r = np.asarray(inputs["W2r"], np.float32)
    b1 = np.asarray(inputs["b1"], np.float32)
    b2 = np.asarray(inputs["b2"], np.float32)
    iota = np.tile(np.arange(P, dtype=np.float32)[None, :], (P, 1)).astype(BF16)
    in_maps = []
    for k in range(n_cores):
        pc = per_core[k]
        in_maps.append({
            "x_all": x_bf,
            "xT": np.ascontiguousarray(x[_gslot_idx(k)].T).astype(BF16),
            "idx_lo": pc["idx_lo"], "idx_hi": pc["idx_hi"],
            "dstv_lo": pc["dstv_lo"], "dstv_hi": pc["dstv_hi"],
            "invdeg": pc["invdeg"],
            "iota": iota,
            "w1lT": np.ascontiguousarray(w1l.T).astype(BF16),
            "w1rT": np.ascontiguousarray(w1r.T).astype(BF16),
            "w2lT": np.ascontiguousarray(w2l.T).astype(BF16),
            "w2rT": np.ascontiguousarray(w2r.T).astype(BF16),
            "b1r": b1[None, :].astype(BF16),
            "b2r": b2[None, :].astype(BF16),
        })
    return in_maps


_CACHE = {}


def _compile(meta):
    key = (meta["Llo"], meta["Lhi"], meta["n_nodes"], meta["rows_per"])
    if key not in _CACHE:
        nc = bacc.Bacc("TRN2", target_bir_lowering=False, debug=False,
                       num_devices=meta["n_cores"], num_swdge_queues=NQ)
        build_graph(nc, meta)
        nc.compile()
        _CACHE[key] = nc
    return _CACHE[key]


def assemble(res, meta):
    out = np.concatenate(
        [np.asarray(res.results[k]["out"]) for k in range(meta["n_cores"])],
        axis=0,
    ).astype(np.float32)
    order = np.concatenate(
        [meta["perm"][_gslot_idx(k)] for k in range(meta["n_cores"])]
    )
    unperm = np.empty_like(out)
    unperm[order] = out
    return unperm


def kernel(**inputs):
    edge_index = np.asarray(inputs["edge_index"])
    meta, per_core = preprocess(edge_index)
    nc = _compile(meta)
    in_maps = make_in_maps(inputs, meta, per_core)
    res = bass_utils.run_bass_kernel_spmd(
        nc, in_maps, core_ids=list(range(meta["n_cores"]))
    )
    return assemble(res, meta)
